# revision 32
# baseline (speedup 1.0000x reference)
"""Trainium2 Bass kernel for nn_MultiHeadAttention (B=4, S=2048, D=1024, H=16).

Sharding: tensor-parallel over heads (2 heads per core, 8 cores). Each core:
  1. Projects Q/K (feature-major, [128 feats x 8192 seq]) and V (seq-major via
     PE transpose, augmented with a ones-column for the softmax denominator).
  2. Computes causal attention for its 8 (batch, head) pairs in bf16 with
     fp32 PSUM accumulation: scoresT = K-chunk @ Q-strip, exp on ACT,
     AV+denominator via one accumulating matmul against [V | 1].
     Softmax normalization is deferred to the end of each batch: the 8
     denominator rows are packed into one [8, 512] tile, inverted with a
     single reciprocal_approx_fast, and broadcast back to 64 partitions via
     small selector matmuls (avoids 3.3us single-lane reciprocals).
  3. A per-batch AllGather (bf16 payload) publishes attention outputs; the
     first three overlap the next batch's compute. Each core then pulls the
     features of its own 1024-row sequence block via an indirect (index-
     driven) DMA gather -- the indices come from a per-core input tensor, so
     the SPMD program stays identical across cores -- and computes its block
     of the output projection in bf16. A chained dummy-matmul "warm keeper"
     spans the final AllGather wait to hold the PE clock at 2.4 GHz.
Host wraps: shards weights (with 1/sqrt(dk) folded into Wq), classifies mask
blocks (skip / keep / masked via unique [128, 512] tiles), and reassembles
the full [4, 2048, 1024] output.
"""

import ml_dtypes
import numpy as np

import concourse.bass as bass
import concourse.bacc as bacc
import concourse.mybir as mybir
import concourse.tile as tile
from concourse.bass_utils import run_bass_kernel_spmd

F32 = mybir.dt.float32
F32R = mybir.dt.float32r
F16 = mybir.dt.float16
BF16 = mybir.dt.bfloat16
AF = mybir.ActivationFunctionType
OP = mybir.AluOpType

B, S, D_MODEL, N_HEADS, D_K = 4, 2048, 1024, 16, 64
N_CORES = 8
HPC = N_HEADS // N_CORES          # heads per core = 2
F = HPC * D_K                     # feature slice per core = 128
SEQ = B * S                       # 8192
S1B = 512                         # query-strip width (scores free dim)
S2B = 128                         # key-block height (scores partition dim)
SP = S // S1B                     # 4 strips per batch
C2 = S // S2B                     # 16 key chunks per batch
KC = D_MODEL // 128               # 8 contraction chunks for projections
SC_GLOBAL = SEQ // S1B            # 16 projection seq strips
SH = SP * HPC                     # strip-head pairs per batch = 8
A_DROP, A_KEEP = -2, -1
A_TRI0 = -3                       # A_TRI0 - j: block valid from col j*128 on,
                                  # first 128-col sub-block lower-triangular
TRIW = 128
BINW = 512                        # one fp32 PSUM bank
GRPW = 1024                       # score-group width (2 fp32 PSUM banks)


def _pack_segs(segs):
    """Bin-pack (i2, kind, qo, w) segments into PSUM-bank-aligned groups.

    Each bin holds <= 512 cols (one fp32 bank). A group is [full bin, bin]
    or [bin] so the packed columns are contiguous (exp reads [0, gw))."""
    bins = []
    for sg in sorted(segs, key=lambda s: -s[3]):
        for bn in bins:
            if bn[0] + sg[3] <= BINW:
                bn[0] += sg[3]
                bn[1].append(sg)
                break
        else:
            bins.append([sg[3], [sg]])
    groups, i = [], 0
    while i < len(bins):
        if bins[i][0] == BINW and i + 1 < len(bins):
            groups.append(bins[i][1] + bins[i + 1][1])
            i += 2
        else:
            groups.append(bins[i][1])
            i += 1
    return groups

_nc_cache = {}


def _build_nc(actions_key, n_masks):
    actions = np.frombuffer(actions_key, dtype=np.int64).reshape(C2, SP)
    nc = bacc.Bacc("TRN2", target_bir_lowering=False, debug=False,
                   num_devices=N_CORES)

    xq = nc.dram_tensor("xq", [D_MODEL, SEQ], BF16, kind="ExternalInput")
    xk = nc.dram_tensor("xk", [D_MODEL, SEQ], BF16, kind="ExternalInput")
    xv = nc.dram_tensor("xv", [D_MODEL, SEQ], BF16, kind="ExternalInput")
    wqkv = nc.dram_tensor("wqkv", [128, KC, 3, F], BF16, kind="ExternalInput")
    b3 = nc.dram_tensor("b3", [F, 3], F32, kind="ExternalInput")
    woT = nc.dram_tensor("woT", [128, KC, KC, 128], BF16, kind="ExternalInput")
    bo = nc.dram_tensor("bo", [128, KC], F32, kind="ExternalInput")
    ident = nc.dram_tensor("ident", [128, 128], BF16, kind="ExternalInput")
    trim = nc.dram_tensor("trim", [TRIW, TRIW], BF16, kind="ExternalInput")
    masks = nc.dram_tensor("masks", [max(n_masks, 1), S2B, S1B], BF16,
                           kind="ExternalInput")

    HH = SH // 2                  # strip-head pairs per half-batch = 4
    sel = nc.dram_tensor("sel", [HH, HH, D_K], F32, kind="ExternalInput")
    oidx = nc.dram_tensor("oidx", [KC, 128, 1], mybir.dt.int32, kind="ExternalInput")
    agin = nc.dram_tensor("agin", [B, 2, F, 2 * S1B], BF16)
    agf = nc.dram_tensor("agf", [B, 2, N_CORES, F, 2 * S1B], BF16,
                         addr_space="Shared")
    out_t = nc.dram_tensor("out_t", [D_MODEL, SEQ // N_CORES], F32,
                           kind="ExternalOutput")

    with tile.TileContext(nc) as tc:
      with tc.tile_pool(name="oproj_w", bufs=1) as opw:
        wo_sb = opw.tile([128, KC, KC, 128], BF16, tag="wo")
        bo_sb = opw.tile([128, KC], F32, tag="bo")
        rhs = opw.tile([128, KC, SEQ // N_CORES], BF16, tag="rhs")
        with (
            tc.tile_pool(name="const", bufs=1) as cst,
            tc.tile_pool(name="persist", bufs=1) as per,
            tc.tile_pool(name="xin", bufs=12) as xin,
            tc.tile_pool(name="vtmp", bufs=2) as vtmp,
            tc.tile_pool(name="probs", bufs=8) as prp,
            tc.tile_pool(name="avkeep", bufs=SH) as avp,
            tc.tile_pool(name="norm", bufs=2) as nrm,
            tc.tile_pool(name="pp_ps", bufs=2, space="PSUM") as pp_ps,
            tc.tile_pool(name="sc_ps", bufs=2, space="PSUM") as sc_ps,
            tc.tile_pool(name="av_ps", bufs=2, space="PSUM") as av_ps,
        ):
            wqkv_sb = cst.tile([128, KC, 3, F], BF16, tag="wqkv")
            nc.sync.dma_start(wqkv_sb[:], wqkv[:])
            b3_sb = cst.tile([F, 3], F32, tag="b3")
            nc.sync.dma_start(b3_sb[:], b3[:])
            id_sb = cst.tile([128, 128], BF16, tag="id")
            nc.sync.dma_start(id_sb[:], ident[:])
            trim_sb = cst.tile([TRIW, TRIW], BF16, tag="trim")
            nc.sync.dma_start(trim_sb[:], trim[:])
            mk_sb = cst.tile([S2B, max(n_masks, 1), S1B], BF16, tag="mk")
            nc.sync.dma_start(mk_sb[:], masks[:].rearrange("n p f -> p n f"))

            qT = per.tile([F, SEQ], BF16, tag="qT")
            kT = per.tile([F, SEQ], BF16, tag="kT")
            # V (seq-major) with per-head ones column: [s2_in_chunk, b, c2, h, dk+1]
            v_aug = per.tile([S2B, B, C2, HPC, D_K + 1], BF16, tag="vaug")
            # ones columns of v_aug never change: set once per batch slot
            for bb in range(B):
                nc.vector.memset(v_aug[:, bb, :, :, D_K:D_K + 1], 1.0)
            # selector for denominator broadcast: sel[k, j, :] = (k == j)
            sel_f = cst.tile([HH, HH, D_K], F32, tag="self")
            nc.sync.dma_start(sel_f[:], sel[:])
            sel_sb = cst.tile([HH, HH, D_K], F32R, tag="sel")
            nc.vector.tensor_copy(sel_sb[:], sel_f[:])
            # per-round output-projection gather staging
            idx_sb = cst.tile([128, KC], mybir.dt.int32, tag="idx")
            nc.sync.dma_start(idx_sb[:], oidx[:].rearrange("g p one -> p (g one)"))

            for b in range(B):
                # --- projections for batch b (per tensor: load 8 k-chunks
                # of the batch as [128, 2048] tiles, project 4 strips) ---
                for t_idx, (name, x_dram) in enumerate(
                        (("q", xq), ("k", xk), ("v", xv))):
                    w_view = wqkv_sb[:, :, t_idx, :]
                    b_view = b3_sb[:, t_idx:t_idx + 1]
                    xts = []
                    for kc in range(KC):
                        xt = xin.tile([128, S], BF16, tag="xt")
                        nc.sync.dma_start(
                            xt[:], x_dram[kc * 128:(kc + 1) * 128,
                                          b * S:(b + 1) * S])
                        xts.append(xt)
                    for sc_local in range(SP):
                        sl = slice(sc_local * S1B, (sc_local + 1) * S1B)
                        gsl = slice(b * S + sc_local * S1B,
                                    b * S + (sc_local + 1) * S1B)
                        ps = pp_ps.tile([128, S1B], F32, tag="pp")
                        for kc in range(KC):
                            nc.tensor.matmul(ps[:], w_view[:, kc, :], xts[kc][:, sl],
                                             start=(kc == 0), stop=(kc == KC - 1))
                        if name == "q":
                            nc.vector.tensor_scalar_add(qT[:, gsl], ps[:], b_view)
                        elif name == "k":
                            nc.vector.tensor_scalar_add(kT[:, gsl], ps[:], b_view)
                        else:
                            vt = vtmp.tile([128, S1B], BF16, tag="vt")
                            nc.vector.tensor_scalar_add(vt[:], ps[:], b_view)
                            for j in range(S1B // 128):
                                tp = pp_ps.tile([128, 128], BF16, tag="pp")
                                nc.tensor.transpose(tp[:], vt[:, j * 128:(j + 1) * 128],
                                                    id_sb[:])
                                c2 = sc_local * (S1B // 128) + j
                                nc.vector.tensor_copy(
                                    v_aug[:, b, c2, :, 0:D_K],
                                    tp[:].rearrange("p (h d) -> p h d", h=HPC))

                # --- attention for batch b, both local heads interleaved ---
                avcs = [None] * HH
                den4 = None
                for i1 in range(SP):
                    if i1 % 2 == 0:
                        den4 = nrm.tile([HH, S1B], F32, tag="den")
                    segs = []  # (i2, kind, q-offset, width)
                    for i2 in range(C2):
                        a = actions[i2, i1]
                        if a == A_DROP:
                            continue
                        if a <= A_TRI0:
                            j = A_TRI0 - a
                            segs.append((i2, "tri", j * TRIW, S1B - j * TRIW))
                        elif a == A_KEEP:
                            segs.append((i2, "keep", 0, S1B))
                        else:
                            segs.append((i2, int(a), 0, S1B))
                    groups = _pack_segs(segs)
                    nseg = len(segs)
                    avs = []
                    for _lh in range(HPC):
                        av_t = av_ps.tile([D_K + 1, S1B], F32, tag="av")
                        avs.append(av_t)
                    seg_base = 0
                    pend = None  # (grp, prs_all, base) awaiting AV emission
                    def emit_av(grp, prs_all, base):
                        for lh in range(HPC):
                            pr, off = prs_all[lh], 0
                            for si, (i2, kind, qo, w) in enumerate(grp):
                                if kind == "tri":
                                    nc.vector.tensor_tensor(
                                        pr[:, off:off + TRIW],
                                        pr[:, off:off + TRIW], trim_sb[:],
                                        OP.mult)
                                elif isinstance(kind, int):
                                    nc.vector.tensor_tensor(
                                        pr[:, off:off + w], pr[:, off:off + w],
                                        mk_sb[:, kind, 0:w], OP.mult)
                                nc.tensor.matmul(
                                    avs[lh][:, qo:qo + w],
                                    v_aug[:, b, i2, lh, :], pr[:, off:off + w],
                                    start=(base + si == 0),
                                    stop=(base + si == nseg - 1))
                                off += w
                    for grp in groups:
                        gw = sum(s[3] for s in grp)
                        sc_ts = [sc_ps.tile([128, GRPW], F32, tag="sc",
                                            name=f"sc_t{lh}")
                                 for lh in range(HPC)]
                        off = 0
                        for (i2, kind, qo, w) in grp:
                            for lh in range(HPC):
                                r0, r1 = lh * D_K, (lh + 1) * D_K
                                nc.tensor.matmul(
                                    sc_ts[lh][:, off:off + w],
                                    kT[r0:r1,
                                       b * S + i2 * S2B: b * S + i2 * S2B + S2B],
                                    qT[r0:r1,
                                       b * S + i1 * S1B + qo:
                                       b * S + i1 * S1B + qo + w],
                                    start=True, stop=True)
                            off += w
                        prs_all = []
                        for lh in range(HPC):
                            pr = prp.tile([128, GRPW], BF16, tag="pr")
                            nc.scalar.activation(pr[:, 0:gw], sc_ts[lh][:, 0:gw],
                                                 AF.Exp)
                            prs_all.append(pr)
                        if pend is not None:
                            emit_av(*pend)
                        pend = (grp, prs_all, seg_base)
                        seg_base += len(grp)
                    if pend is not None:
                        emit_av(*pend)

                    # copy out of PSUM immediately to release the banks;
                    # stash the denominator row into the batch-wide [8, 512]
                    # tile for one batched reciprocal at batch end
                    for lh in range(HPC):
                        sh = (i1 % 2) * HPC + lh
                        avc = avp.tile([D_K + 1, S1B], F32, tag="avc")
                        nc.vector.tensor_copy(avc[:], avs[lh][:])
                        # engine ops can't write partition base sh (unaligned);
                        # an SBUF->SBUF DMA can
                        nc.sync.dma_start(den4[sh:sh + 1, :],
                                          avc[D_K:D_K + 1, :])
                        avcs[sh] = avc

                    if i1 % 2 == 1:
                        # --- softmax normalization + AllGather for this
                        # half-batch (strips i1-1, i1) ---
                        hh = i1 // 2
                        rcp = nrm.tile([HH, S1B], F32, tag="rcp")
                        nc.vector.reciprocal_approx_fast(rcp[:], den4[:])
                        rcpr = nrm.tile([HH, S1B], F32R, tag="rcpr")
                        nc.vector.tensor_copy(rcpr[:], rcp[:])
                        for sh in range(HH):
                            si1, lh = i1 - 1 + sh // HPC, sh % HPC
                            bc_ps = av_ps.tile([D_K, S1B], F32, tag="av")
                            nc.tensor.matmul(bc_ps[:], sel_sb[:, sh, :],
                                             rcpr[:], start=True, stop=True)
                            ob = nrm.tile([D_K, S1B], BF16, tag="ob", bufs=4)
                            nc.vector.tensor_tensor(ob[:], avcs[sh][0:D_K, :],
                                                    bc_ps[:], OP.mult)
                            nc.sync.dma_start(
                                agin[b, hh, lh * D_K:(lh + 1) * D_K,
                                     (si1 % 2) * S1B:(si1 % 2) * S1B + S1B],
                                ob[:])
                        nc.gpsimd.collective_compute(
                            "AllGather", OP.bypass,
                            ins=[agin[b, hh]], outs=[agf[b, hh]],
                            replica_groups=[list(range(N_CORES))])

                # gather this round's 2 feature groups for the output
                # projection (indices are per-core inputs; the last round's
                # gather only waits on this round's AllGathers)
                agf_rows = agf[:].rearrange("b hh g p c -> (b hh g p) c")
                for t in range(2):
                    nc.gpsimd.indirect_dma_start(
                        out=rhs[:, 2 * b + t, :], out_offset=None,
                        in_=agf_rows,
                        in_offset=bass.IndirectOffsetOnAxis(
                            ap=idx_sb[:, 2 * b + t:2 * b + t + 1], axis=0))



        nc.sync.dma_start(wo_sb[:], woT[:])
        nc.sync.dma_start(bo_sb[:], bo[:])
        with (
            tc.tile_pool(name="oproj", bufs=1) as opr,
            tc.tile_pool(name="ob_sb", bufs=3) as obp,
            tc.tile_pool(name="op_ps", bufs=2, space="PSUM") as op_ps,
        ):
            # warm-keeper: chained dummy matmuls span the AllGather wait so
            # the PE clock stays at 2.4GHz for the output projection
            wsb = opr.tile([128, S1B], BF16, tag="wsb")
            nc.vector.tensor_copy(wsb[:], wo_sb[:, 0, 0:4, :].rearrange("p a f -> p (a f)"))
            for _ in range(22):
                wps = op_ps.tile([128, S1B], F32, tag="op")
                nc.tensor.matmul(wps[:], wo_sb[:, 0, 0, :], wsb[:],
                                 start=True, stop=True)
                wsb = opr.tile([128, S1B], BF16, tag="wsb")
                nc.vector.tensor_copy(wsb[:], wps[:])
            n_sc2 = (SEQ // N_CORES) // S1B
            for dc in range(KC):
                for sc2 in range(n_sc2):
                    ps = op_ps.tile([128, S1B], F32, tag="op")
                    for kc in range(KC):
                        nc.tensor.matmul(
                            ps[:], wo_sb[:, kc, dc, :],
                            rhs[:, kc, sc2 * S1B:(sc2 + 1) * S1B],
                            start=(kc == 0), stop=(kc == KC - 1))
                    ob = obp.tile([128, S1B], F32, tag="obt")
                    nc.vector.tensor_scalar_add(ob[:], ps[:], bo_sb[:, dc:dc + 1])
                    nc.sync.dma_start(
                        out_t[dc * 128:(dc + 1) * 128,
                              sc2 * S1B:(sc2 + 1) * S1B], ob[:])

    nc.finalize()
    return nc


def _classify_mask(mask):
    """Block-classify mask[0,0] on the scoresT grid: per (key-chunk i2,
    query-strip i1) -> drop / keep / index of a unique [128, 512] 0/1 tile."""
    m2 = np.asarray(mask)[0, 0] != 0  # [S, S], m2[q, k]
    actions = np.full((C2, SP), A_DROP, dtype=np.int64)
    uniq, tiles = {}, []
    qs = np.arange(S1B)[None, :]
    ks = np.arange(S2B)[:, None]
    for i2 in range(C2):
        for i1 in range(SP):
            blk = m2[i1 * S1B:(i1 + 1) * S1B, i2 * S2B:(i2 + 1) * S2B].T
            if blk.all():
                actions[i2, i1] = A_KEEP
            elif blk.any():
                j = i2 - (S1B // S2B) * i1
                if 0 <= j < S1B // S2B and np.array_equal(
                        blk, qs >= j * TRIW + ks):
                    # causal staircase: valid from col j*128 on, leading
                    # 128-col sub-block lower-triangular
                    actions[i2, i1] = A_TRI0 - j
                    continue
                key = blk.tobytes()
                if key not in uniq:
                    uniq[key] = len(tiles)
                    tiles.append(np.ascontiguousarray(blk).astype(ml_dtypes.bfloat16))
                actions[i2, i1] = uniq[key]
    arr = (np.stack(tiles) if tiles
           else np.zeros((1, S2B, S1B), dtype=ml_dtypes.bfloat16))
    return actions, arr


def _prep(inputs):
    q = np.asarray(inputs["query"], dtype=np.float32).reshape(SEQ, D_MODEL)
    k = np.asarray(inputs["key"], dtype=np.float32).reshape(SEQ, D_MODEL)
    v = np.asarray(inputs["value"], dtype=np.float32).reshape(SEQ, D_MODEL)
    bf = ml_dtypes.bfloat16
    xq = np.ascontiguousarray(q.T).astype(bf)
    xk = np.ascontiguousarray(k.T).astype(bf)
    xv = np.ascontiguousarray(v.T).astype(bf)

    Wq = np.asarray(inputs["Wq"], dtype=np.float32)
    Wk = np.asarray(inputs["Wk"], dtype=np.float32)
    Wv = np.asarray(inputs["Wv"], dtype=np.float32)
    Wo = np.asarray(inputs["Wo"], dtype=np.float32)
    bq = np.asarray(inputs["bq"], dtype=np.float32)
    bk = np.asarray(inputs["bk"], dtype=np.float32)
    bv = np.asarray(inputs["bv"], dtype=np.float32)
    bo = np.asarray(inputs["bo"], dtype=np.float32)

    scale = 1.0 / np.sqrt(D_K)
    actions, mask_tiles = _classify_mask(inputs["mask"])

    # exp-overflow guard for the no-max-subtract softmax (Cauchy-Schwarz bound)
    qn = q @ Wq.T + bq
    kn = k @ Wk.T + bk
    qmax = np.linalg.norm(qn.reshape(SEQ, N_HEADS, D_K), axis=-1).max()
    kmax = np.linalg.norm(kn.reshape(SEQ, N_HEADS, D_K), axis=-1).max()
    assert scale * qmax * kmax < 80.0, "score bound too large for exp without max-subtraction"

    WoT = np.ascontiguousarray(Wo.T).astype(np.float32)
    shared = {
        "bo": np.ascontiguousarray(bo.reshape(KC, 128).T),
        "ident": np.eye(128, dtype=np.float32).astype(bf),
        "trim": np.ascontiguousarray(
            (np.arange(TRIW)[None, :] >= np.arange(TRIW)[:, None])
            .astype(np.float32)).astype(bf),
        "masks": mask_tiles,
        "sel": np.ascontiguousarray(
            (np.eye(SH // 2, dtype=np.float32)[:, :, None]
             * np.ones((1, 1, D_K), dtype=np.float32))),
    }
    in_maps = []
    for c in range(N_CORES):
        sl = slice(c * F, (c + 1) * F)
        m = dict(shared)
        bc = c // 2
        order = [(bc + k) % B for k in range(B)]
        # batch-slot reorder: slot k of this core's inputs holds batch
        # (bc + k) % B, so the core's own batch completes first and each
        # AllGather round delivers 2 feature groups of its out-proj rhs
        for nm, arr in (("xq", xq), ("xk", xk), ("xv", xv)):
            m[nm] = np.ascontiguousarray(np.concatenate(
                [arr[:, ob * S:(ob + 1) * S] for ob in order], axis=1))
        wq3 = np.concatenate(
            [(Wq[sl] * scale).T, Wk[sl].T, Wv[sl].T], axis=1)  # [D, 3F]
        m["wqkv"] = np.ascontiguousarray(
            wq3.reshape(KC, 128, 3, F).transpose(1, 0, 2, 3)).astype(bf)
        m["b3"] = np.ascontiguousarray(np.stack(
            [bq[sl] * scale, bk[sl], bv[sl]], axis=1).astype(np.float32))
        # out-proj rhs slot 2k+t holds feature group g = 2a+t of round k,
        # a = (bc - k) % 4; permute WoT's contraction blocks to match
        slot_g = []
        for k in range(B):
            a = (bc - k) % B
            slot_g += [2 * a, 2 * a + 1]
        woT_p = np.concatenate([WoT[g * 128:(g + 1) * 128, :] for g in slot_g],
                               axis=0)  # [D, D] slot-ordered rows
        m["woT"] = np.ascontiguousarray(
            woT_p.reshape(KC, 128, KC, 128).transpose(1, 0, 2, 3)).astype(bf)
        # gather indices: agf rows are (k, hh, g, p) ->
        # ((k*2 + hh)*8 + g)*128 + p
        pp = np.arange(128)
        idx = np.empty((KC, 128), dtype=np.int32)
        for k in range(B):
            a = (bc - k) % B
            for t in range(2):
                g = 2 * a + t
                idx[2 * k + t] = (((k * 2 + (c % 2)) * N_CORES) + g) * 128 + pp
        m["oidx"] = np.ascontiguousarray(idx.reshape(KC, 128, 1))
        in_maps.append(m)
    return in_maps, actions, mask_tiles


def _run(inputs, trace=False, trace_cores=None):
    in_maps, actions, mask_tiles = _prep(inputs)
    key = (actions.tobytes(), len(mask_tiles))
    if key not in _nc_cache:
        _nc_cache[key] = _build_nc(key[0], key[1])
    nc = _nc_cache[key]
    res = run_bass_kernel_spmd(nc, in_maps, list(range(N_CORES)),
                               trace=trace, trace_cores=trace_cores)
    blk = SEQ // N_CORES
    out = np.empty((SEQ, D_MODEL), dtype=np.float32)
    for c in range(N_CORES):
        out[c * blk:(c + 1) * blk] = res.results[c]["out_t"].T
    return out.reshape(B, S, D_MODEL), res


def kernel(**inputs) -> np.ndarray:
    out, _ = _run(inputs)
    return out


# revision 34
# speedup vs baseline: 1.1375x; 1.1375x over previous
"""Trainium2 Bass kernel for nn_MultiHeadAttention (B=4, S=2048, D=1024, H=16).

Sharding: tensor-parallel over heads (2 heads per core, 8 cores). Each core:
  1. Projects Q/K (feature-major, [128 feats x 8192 seq]) and V (seq-major via
     PE transpose, augmented with a ones-column for the softmax denominator).
  2. Computes causal attention for its 8 (batch, head) pairs in bf16 with
     fp32 PSUM accumulation: scoresT = K-chunk @ Q-strip, exp on ACT,
     AV+denominator via one accumulating matmul against [V | 1].
     Softmax normalization is deferred to the end of each batch: the 8
     denominator rows are packed into one [8, 512] tile, inverted with a
     single reciprocal_approx_fast, and broadcast back to 64 partitions via
     small selector matmuls (avoids 3.3us single-lane reciprocals).
  3. A per-batch AllGather (bf16 payload) publishes attention outputs; the
     first three overlap the next batch's compute. Each core then pulls the
     features of its own 1024-row sequence block via an indirect (index-
     driven) DMA gather -- the indices come from a per-core input tensor, so
     the SPMD program stays identical across cores -- and computes its block
     of the output projection in bf16. A chained dummy-matmul "warm keeper"
     spans the final AllGather wait to hold the PE clock at 2.4 GHz.
Host wraps: shards weights (with 1/sqrt(dk) folded into Wq), classifies mask
blocks (skip / keep / masked via unique [128, 512] tiles), and reassembles
the full [4, 2048, 1024] output.
"""

import ml_dtypes
import numpy as np

import concourse.bass as bass
import concourse.bacc as bacc
import concourse.mybir as mybir
import concourse.tile as tile
from concourse.bass_utils import run_bass_kernel_spmd

F32 = mybir.dt.float32
F32R = mybir.dt.float32r
F16 = mybir.dt.float16
BF16 = mybir.dt.bfloat16
AF = mybir.ActivationFunctionType
OP = mybir.AluOpType

B, S, D_MODEL, N_HEADS, D_K = 4, 2048, 1024, 16, 64
N_CORES = 8
HPC = N_HEADS // N_CORES          # heads per core = 2
F = HPC * D_K                     # feature slice per core = 128
SEQ = B * S                       # 8192
S1B = 512                         # query-strip width (scores free dim)
S2B = 128                         # key-block height (scores partition dim)
SP = S // S1B                     # 4 strips per batch
C2 = S // S2B                     # 16 key chunks per batch
KC = D_MODEL // 128               # 8 contraction chunks for projections
SC_GLOBAL = SEQ // S1B            # 16 projection seq strips
SH = SP * HPC                     # strip-head pairs per batch = 8
A_DROP, A_KEEP = -2, -1
A_TRI0 = -3                       # A_TRI0 - j: block valid from col j*128 on,
                                  # first 128-col sub-block lower-triangular
TRIW = 128
BINW = 512                        # one fp32 PSUM bank
GRPW = 1024                       # score-group width (2 fp32 PSUM banks)


def _pack_segs(segs):
    """Bin-pack (i2, kind, qo, w) segments into PSUM-bank-aligned groups.

    Each bin holds <= 512 cols (one fp32 bank). A group is [full bin, bin]
    or [bin] so the packed columns are contiguous (exp reads [0, gw))."""
    bins = []
    for sg in sorted(segs, key=lambda s: -s[3]):
        for bn in bins:
            if bn[0] + sg[3] <= BINW:
                bn[0] += sg[3]
                bn[1].append(sg)
                break
        else:
            bins.append([sg[3], [sg]])
    groups, i = [], 0
    while i < len(bins):
        if bins[i][0] == BINW and i + 1 < len(bins):
            groups.append(bins[i][1] + bins[i + 1][1])
            i += 2
        else:
            groups.append(bins[i][1])
            i += 1
    return groups

_nc_cache = {}


def _build_nc(actions_key, n_masks):
    actions = np.frombuffer(actions_key, dtype=np.int64).reshape(C2, SP)
    nc = bacc.Bacc("TRN2", target_bir_lowering=False, debug=False,
                   num_devices=N_CORES)

    xq = nc.dram_tensor("xq", [D_MODEL, SEQ], BF16, kind="ExternalInput")
    xk = nc.dram_tensor("xk", [D_MODEL, SEQ], BF16, kind="ExternalInput")
    xv = nc.dram_tensor("xv", [D_MODEL, SEQ], BF16, kind="ExternalInput")
    wqkv = nc.dram_tensor("wqkv", [128, KC, 3, F], BF16, kind="ExternalInput")
    b3 = nc.dram_tensor("b3", [F, 3], F32, kind="ExternalInput")
    woT = nc.dram_tensor("woT", [128, KC, KC, 128], BF16, kind="ExternalInput")
    bo = nc.dram_tensor("bo", [128, KC], F32, kind="ExternalInput")
    ident = nc.dram_tensor("ident", [128, 128], BF16, kind="ExternalInput")
    trim = nc.dram_tensor("trim", [TRIW, TRIW], BF16, kind="ExternalInput")
    masks = nc.dram_tensor("masks", [max(n_masks, 1), S2B, S1B], BF16,
                           kind="ExternalInput")

    HH = SH // 2                  # strip-head pairs per half-batch = 4
    sel = nc.dram_tensor("sel", [HH, HH, D_K], F32, kind="ExternalInput")
    oidx = nc.dram_tensor("oidx", [KC, 128, 1], mybir.dt.int32, kind="ExternalInput")
    agin = nc.dram_tensor("agin", [B, 2, F, 2 * S1B], BF16)
    agf = nc.dram_tensor("agf", [B, 2, N_CORES, F, 2 * S1B], BF16,
                         addr_space="Shared")
    out_t = nc.dram_tensor("out_t", [D_MODEL, SEQ // N_CORES], F32,
                           kind="ExternalOutput")

    with tile.TileContext(nc) as tc:
      with tc.tile_pool(name="oproj_w", bufs=1) as opw:
        wo_sb = opw.tile([128, KC, KC, 128], BF16, tag="wo")
        bo_sb = opw.tile([128, KC], F32, tag="bo")
        rhs = opw.tile([128, KC, SEQ // N_CORES], BF16, tag="rhs")
        with (
            tc.tile_pool(name="const", bufs=1) as cst,
            tc.tile_pool(name="persist", bufs=1) as per,
            tc.tile_pool(name="xin", bufs=14) as xin,
            tc.tile_pool(name="vtmp", bufs=2) as vtmp,
            tc.tile_pool(name="probs", bufs=8) as prp,
            tc.tile_pool(name="avkeep", bufs=SH) as avp,
            tc.tile_pool(name="norm", bufs=2) as nrm,
            tc.tile_pool(name="pp_ps", bufs=2, space="PSUM") as pp_ps,
            tc.tile_pool(name="sc_ps", bufs=2, space="PSUM") as sc_ps,
            tc.tile_pool(name="av_ps", bufs=2, space="PSUM") as av_ps,
        ):
            wqkv_sb = cst.tile([128, KC, 3, F], BF16, tag="wqkv")
            nc.sync.dma_start(wqkv_sb[:], wqkv[:])
            b3_sb = cst.tile([F, 3], F32, tag="b3")
            nc.sync.dma_start(b3_sb[:], b3[:])
            id_sb = cst.tile([128, 128], BF16, tag="id")
            nc.sync.dma_start(id_sb[:], ident[:])
            trim_sb = cst.tile([TRIW, TRIW], BF16, tag="trim")
            nc.sync.dma_start(trim_sb[:], trim[:])
            mk_sb = cst.tile([S2B, max(n_masks, 1), S1B], BF16, tag="mk")
            nc.sync.dma_start(mk_sb[:], masks[:].rearrange("n p f -> p n f"))

            qT = per.tile([F, SEQ], BF16, tag="qT")
            kT = per.tile([F, SEQ], BF16, tag="kT")
            # V (seq-major) with per-head ones column: [s2_in_chunk, b, c2, h, dk+1]
            v_aug = per.tile([S2B, B, C2, HPC, D_K + 1], BF16, tag="vaug")
            # ones columns of v_aug never change: set once per batch slot
            for bb in range(B):
                nc.vector.memset(v_aug[:, bb, :, :, D_K:D_K + 1], 1.0)
            # selector for denominator broadcast: sel[k, j, :] = (k == j)
            sel_f = cst.tile([HH, HH, D_K], F32, tag="self")
            nc.sync.dma_start(sel_f[:], sel[:])
            sel_sb = cst.tile([HH, HH, D_K], F32R, tag="sel")
            nc.vector.tensor_copy(sel_sb[:], sel_f[:])
            # per-round output-projection gather staging
            idx_sb = cst.tile([128, KC], mybir.dt.int32, tag="idx")
            nc.sync.dma_start(idx_sb[:], oidx[:].rearrange("g p one -> p (g one)"))

            for b in range(B):
                # --- projections for batch b (per tensor: load 8 k-chunks
                # of the batch as [128, 2048] tiles, project 4 strips) ---
                for t_idx, (name, x_dram) in enumerate(
                        (("q", xq), ("k", xk), ("v", xv))):
                    w_view = wqkv_sb[:, :, t_idx, :]
                    b_view = b3_sb[:, t_idx:t_idx + 1]
                    xts = []
                    for kc in range(KC):
                        xt = xin.tile([128, S], BF16, tag="xt")
                        nc.sync.dma_start(
                            xt[:], x_dram[kc * 128:(kc + 1) * 128,
                                          b * S:(b + 1) * S])
                        xts.append(xt)
                    for sc_local in range(SP):
                        sl = slice(sc_local * S1B, (sc_local + 1) * S1B)
                        gsl = slice(b * S + sc_local * S1B,
                                    b * S + (sc_local + 1) * S1B)
                        ps = pp_ps.tile([128, S1B], F32, tag="pp")
                        for kc in range(KC):
                            nc.tensor.matmul(ps[:], w_view[:, kc, :], xts[kc][:, sl],
                                             start=(kc == 0), stop=(kc == KC - 1))
                        if name == "q":
                            nc.vector.tensor_scalar_add(qT[:, gsl], ps[:], b_view)
                        elif name == "k":
                            nc.vector.tensor_scalar_add(kT[:, gsl], ps[:], b_view)
                        else:
                            vt = vtmp.tile([128, S1B], BF16, tag="vt")
                            nc.vector.tensor_scalar_add(vt[:], ps[:], b_view)
                            for j in range(S1B // 128):
                                tp = pp_ps.tile([128, 128], BF16, tag="pp")
                                nc.tensor.transpose(tp[:], vt[:, j * 128:(j + 1) * 128],
                                                    id_sb[:])
                                c2 = sc_local * (S1B // 128) + j
                                nc.vector.tensor_copy(
                                    v_aug[:, b, c2, :, 0:D_K],
                                    tp[:].rearrange("p (h d) -> p h d", h=HPC))

                # --- attention for batch b, both local heads interleaved ---
                avcs = [None] * HH
                den4 = None
                for i1 in range(SP):
                    if i1 % 2 == 0:
                        den4 = nrm.tile([HH, S1B], F32, tag="den")
                    segs = []  # (i2, kind, q-offset, width)
                    for i2 in range(C2):
                        a = actions[i2, i1]
                        if a == A_DROP:
                            continue
                        if a <= A_TRI0:
                            j = A_TRI0 - a
                            segs.append((i2, "tri", j * TRIW, S1B - j * TRIW))
                        elif a == A_KEEP:
                            segs.append((i2, "keep", 0, S1B))
                        else:
                            segs.append((i2, int(a), 0, S1B))
                    groups = _pack_segs(segs)
                    nseg = len(segs)
                    avs = []
                    for _lh in range(HPC):
                        av_t = av_ps.tile([D_K + 1, S1B], F32, tag="av")
                        avs.append(av_t)
                    seg_base = 0
                    pend = None  # (grp, prs_all, base) awaiting AV emission
                    def emit_av(grp, prs_all, base):
                        for lh in range(HPC):
                            pr, off = prs_all[lh], 0
                            for si, (i2, kind, qo, w) in enumerate(grp):
                                if kind == "tri":
                                    nc.vector.tensor_tensor(
                                        pr[:, off:off + TRIW],
                                        pr[:, off:off + TRIW], trim_sb[:],
                                        OP.mult)
                                elif isinstance(kind, int):
                                    nc.vector.tensor_tensor(
                                        pr[:, off:off + w], pr[:, off:off + w],
                                        mk_sb[:, kind, 0:w], OP.mult)
                                nc.tensor.matmul(
                                    avs[lh][:, qo:qo + w],
                                    v_aug[:, b, i2, lh, :], pr[:, off:off + w],
                                    start=(base + si == 0),
                                    stop=(base + si == nseg - 1))
                                off += w
                    for grp in groups:
                        gw = sum(s[3] for s in grp)
                        sc_ts = [sc_ps.tile([128, GRPW], F32, tag="sc",
                                            name=f"sc_t{lh}")
                                 for lh in range(HPC)]
                        off = 0
                        for (i2, kind, qo, w) in grp:
                            for lh in range(HPC):
                                r0, r1 = lh * D_K, (lh + 1) * D_K
                                nc.tensor.matmul(
                                    sc_ts[lh][:, off:off + w],
                                    kT[r0:r1,
                                       b * S + i2 * S2B: b * S + i2 * S2B + S2B],
                                    qT[r0:r1,
                                       b * S + i1 * S1B + qo:
                                       b * S + i1 * S1B + qo + w],
                                    start=True, stop=True)
                            off += w
                        prs_all = []
                        for lh in range(HPC):
                            pr = prp.tile([128, GRPW], BF16, tag="pr")
                            nc.scalar.activation(pr[:, 0:gw], sc_ts[lh][:, 0:gw],
                                                 AF.Exp)
                            prs_all.append(pr)
                        if pend is not None:
                            emit_av(*pend)
                        pend = (grp, prs_all, seg_base)
                        seg_base += len(grp)
                    if pend is not None:
                        emit_av(*pend)

                    # copy out of PSUM immediately to release the banks;
                    # stash the denominator row into the batch-wide [8, 512]
                    # tile for one batched reciprocal at batch end
                    for lh in range(HPC):
                        sh = (i1 % 2) * HPC + lh
                        avc = avp.tile([D_K + 1, S1B], F32, tag="avc")
                        nc.vector.tensor_copy(avc[:], avs[lh][:])
                        # engine ops can't write partition base sh (unaligned);
                        # an SBUF->SBUF DMA can
                        nc.sync.dma_start(den4[sh:sh + 1, :],
                                          avc[D_K:D_K + 1, :])
                        avcs[sh] = avc

                    if i1 % 2 == 1:
                        # --- softmax normalization + AllGather for this
                        # half-batch (strips i1-1, i1) ---
                        hh = i1 // 2
                        rcp = nrm.tile([HH, S1B], F32, tag="rcp")
                        nc.vector.reciprocal_approx_fast(rcp[:], den4[:])
                        rcpr = nrm.tile([HH, S1B], F32R, tag="rcpr")
                        nc.vector.tensor_copy(rcpr[:], rcp[:])
                        for sh in range(HH):
                            si1, lh = i1 - 1 + sh // HPC, sh % HPC
                            bc_ps = av_ps.tile([D_K, S1B], F32, tag="av")
                            nc.tensor.matmul(bc_ps[:], sel_sb[:, sh, :],
                                             rcpr[:], start=True, stop=True)
                            ob = nrm.tile([D_K, S1B], BF16, tag="ob", bufs=4)
                            nc.vector.tensor_tensor(ob[:], avcs[sh][0:D_K, :],
                                                    bc_ps[:], OP.mult)
                            nc.sync.dma_start(
                                agin[b, hh, lh * D_K:(lh + 1) * D_K,
                                     (si1 % 2) * S1B:(si1 % 2) * S1B + S1B],
                                ob[:])
                        nc.gpsimd.collective_compute(
                            "AllGather", OP.bypass,
                            ins=[agin[b, hh]], outs=[agf[b, hh]],
                            replica_groups=[list(range(N_CORES))])

                # gather this round's 2 feature groups for the output
                # projection (indices are per-core inputs; the last round's
                # gather only waits on this round's AllGathers)
                agf_rows = agf[:].rearrange("b hh g p c -> (b hh g p) c")
                for t in range(2):
                    nc.gpsimd.indirect_dma_start(
                        out=rhs[:, 2 * b + t, :], out_offset=None,
                        in_=agf_rows,
                        in_offset=bass.IndirectOffsetOnAxis(
                            ap=idx_sb[:, 2 * b + t:2 * b + t + 1], axis=0))



        nc.sync.dma_start(wo_sb[:], woT[:])
        nc.sync.dma_start(bo_sb[:], bo[:])
        with (
            tc.tile_pool(name="ob_sb", bufs=3) as obp,
            tc.tile_pool(name="op_ps", bufs=8, space="PSUM") as op_ps,
        ):
            # output projection with split accumulation: kc slots 0-5 arrive
            # with AllGather rounds 0-2, so their partial sums run on the PE
            # while the final AllGather is still in flight (keeps PE warm);
            # slots 6-7 (round 3) finish each accumulator afterwards.
            n_sc2 = (SEQ // N_CORES) // S1B
            for sc2 in range(n_sc2):
                pss = []
                for dc in range(KC):
                    ps = op_ps.tile([128, S1B], F32, tag="op", name=f"ps{dc}")
                    pss.append(ps)
                for dc in range(KC):
                    for kc in range(KC - 2):
                        nc.tensor.matmul(
                            pss[dc][:], wo_sb[:, kc, dc, :],
                            rhs[:, kc, sc2 * S1B:(sc2 + 1) * S1B],
                            start=(kc == 0), stop=False)
                for dc in range(KC):
                    for kc in (KC - 2, KC - 1):
                        nc.tensor.matmul(
                            pss[dc][:], wo_sb[:, kc, dc, :],
                            rhs[:, kc, sc2 * S1B:(sc2 + 1) * S1B],
                            start=False, stop=(kc == KC - 1))
                    ob = obp.tile([128, S1B], F32, tag="obt")
                    nc.vector.tensor_scalar_add(ob[:], pss[dc][:],
                                                bo_sb[:, dc:dc + 1])
                    nc.sync.dma_start(
                        out_t[dc * 128:(dc + 1) * 128,
                              sc2 * S1B:(sc2 + 1) * S1B], ob[:])

    nc.finalize()
    return nc


def _classify_mask(mask):
    """Block-classify mask[0,0] on the scoresT grid: per (key-chunk i2,
    query-strip i1) -> drop / keep / index of a unique [128, 512] 0/1 tile."""
    m2 = np.asarray(mask)[0, 0] != 0  # [S, S], m2[q, k]
    actions = np.full((C2, SP), A_DROP, dtype=np.int64)
    uniq, tiles = {}, []
    qs = np.arange(S1B)[None, :]
    ks = np.arange(S2B)[:, None]
    for i2 in range(C2):
        for i1 in range(SP):
            blk = m2[i1 * S1B:(i1 + 1) * S1B, i2 * S2B:(i2 + 1) * S2B].T
            if blk.all():
                actions[i2, i1] = A_KEEP
            elif blk.any():
                j = i2 - (S1B // S2B) * i1
                if 0 <= j < S1B // S2B and np.array_equal(
                        blk, qs >= j * TRIW + ks):
                    # causal staircase: valid from col j*128 on, leading
                    # 128-col sub-block lower-triangular
                    actions[i2, i1] = A_TRI0 - j
                    continue
                key = blk.tobytes()
                if key not in uniq:
                    uniq[key] = len(tiles)
                    tiles.append(np.ascontiguousarray(blk).astype(ml_dtypes.bfloat16))
                actions[i2, i1] = uniq[key]
    arr = (np.stack(tiles) if tiles
           else np.zeros((1, S2B, S1B), dtype=ml_dtypes.bfloat16))
    return actions, arr


def _prep(inputs):
    q = np.asarray(inputs["query"], dtype=np.float32).reshape(SEQ, D_MODEL)
    k = np.asarray(inputs["key"], dtype=np.float32).reshape(SEQ, D_MODEL)
    v = np.asarray(inputs["value"], dtype=np.float32).reshape(SEQ, D_MODEL)
    bf = ml_dtypes.bfloat16
    xq = np.ascontiguousarray(q.T).astype(bf)
    xk = np.ascontiguousarray(k.T).astype(bf)
    xv = np.ascontiguousarray(v.T).astype(bf)

    Wq = np.asarray(inputs["Wq"], dtype=np.float32)
    Wk = np.asarray(inputs["Wk"], dtype=np.float32)
    Wv = np.asarray(inputs["Wv"], dtype=np.float32)
    Wo = np.asarray(inputs["Wo"], dtype=np.float32)
    bq = np.asarray(inputs["bq"], dtype=np.float32)
    bk = np.asarray(inputs["bk"], dtype=np.float32)
    bv = np.asarray(inputs["bv"], dtype=np.float32)
    bo = np.asarray(inputs["bo"], dtype=np.float32)

    scale = 1.0 / np.sqrt(D_K)
    actions, mask_tiles = _classify_mask(inputs["mask"])

    # exp-overflow guard for the no-max-subtract softmax (Cauchy-Schwarz bound)
    qn = q @ Wq.T + bq
    kn = k @ Wk.T + bk
    qmax = np.linalg.norm(qn.reshape(SEQ, N_HEADS, D_K), axis=-1).max()
    kmax = np.linalg.norm(kn.reshape(SEQ, N_HEADS, D_K), axis=-1).max()
    assert scale * qmax * kmax < 80.0, "score bound too large for exp without max-subtraction"

    WoT = np.ascontiguousarray(Wo.T).astype(np.float32)
    shared = {
        "bo": np.ascontiguousarray(bo.reshape(KC, 128).T),
        "ident": np.eye(128, dtype=np.float32).astype(bf),
        "trim": np.ascontiguousarray(
            (np.arange(TRIW)[None, :] >= np.arange(TRIW)[:, None])
            .astype(np.float32)).astype(bf),
        "masks": mask_tiles,
        "sel": np.ascontiguousarray(
            (np.eye(SH // 2, dtype=np.float32)[:, :, None]
             * np.ones((1, 1, D_K), dtype=np.float32))),
    }
    in_maps = []
    for c in range(N_CORES):
        sl = slice(c * F, (c + 1) * F)
        m = dict(shared)
        bc = c // 2
        order = [(bc + k) % B for k in range(B)]
        # batch-slot reorder: slot k of this core's inputs holds batch
        # (bc + k) % B, so the core's own batch completes first and each
        # AllGather round delivers 2 feature groups of its out-proj rhs
        for nm, arr in (("xq", xq), ("xk", xk), ("xv", xv)):
            m[nm] = np.ascontiguousarray(np.concatenate(
                [arr[:, ob * S:(ob + 1) * S] for ob in order], axis=1))
        wq3 = np.concatenate(
            [(Wq[sl] * scale).T, Wk[sl].T, Wv[sl].T], axis=1)  # [D, 3F]
        m["wqkv"] = np.ascontiguousarray(
            wq3.reshape(KC, 128, 3, F).transpose(1, 0, 2, 3)).astype(bf)
        m["b3"] = np.ascontiguousarray(np.stack(
            [bq[sl] * scale, bk[sl], bv[sl]], axis=1).astype(np.float32))
        # out-proj rhs slot 2k+t holds feature group g = 2a+t of round k,
        # a = (bc - k) % 4; permute WoT's contraction blocks to match
        slot_g = []
        for k in range(B):
            a = (bc - k) % B
            slot_g += [2 * a, 2 * a + 1]
        woT_p = np.concatenate([WoT[g * 128:(g + 1) * 128, :] for g in slot_g],
                               axis=0)  # [D, D] slot-ordered rows
        m["woT"] = np.ascontiguousarray(
            woT_p.reshape(KC, 128, KC, 128).transpose(1, 0, 2, 3)).astype(bf)
        # gather indices: agf rows are (k, hh, g, p) ->
        # ((k*2 + hh)*8 + g)*128 + p
        pp = np.arange(128)
        idx = np.empty((KC, 128), dtype=np.int32)
        for k in range(B):
            a = (bc - k) % B
            for t in range(2):
                g = 2 * a + t
                idx[2 * k + t] = (((k * 2 + (c % 2)) * N_CORES) + g) * 128 + pp
        m["oidx"] = np.ascontiguousarray(idx.reshape(KC, 128, 1))
        in_maps.append(m)
    return in_maps, actions, mask_tiles


def _run(inputs, trace=False, trace_cores=None):
    in_maps, actions, mask_tiles = _prep(inputs)
    key = (actions.tobytes(), len(mask_tiles))
    if key not in _nc_cache:
        _nc_cache[key] = _build_nc(key[0], key[1])
    nc = _nc_cache[key]
    res = run_bass_kernel_spmd(nc, in_maps, list(range(N_CORES)),
                               trace=trace, trace_cores=trace_cores)
    blk = SEQ // N_CORES
    out = np.empty((SEQ, D_MODEL), dtype=np.float32)
    for c in range(N_CORES):
        out[c * blk:(c + 1) * blk] = res.results[c]["out_t"].T
    return out.reshape(B, S, D_MODEL), res


def kernel(**inputs) -> np.ndarray:
    out, _ = _run(inputs)
    return out
